# revision 39
# baseline (speedup 1.0000x reference)
"""Trainium2 Bass kernel for nn_Mixer2dTriUKAN_66417374265858.

Mathematical simplification: in gcn_spatial the adjacency enters only as
s = sum(softmax(P), axis=-1) == 1, so the entire FFT/prob_distance/softmax
branch cancels and gcn_spatial(x, a, w, b) == gelu(x @ (w1+w2+w3).T + b)
where w = [w1|w2|w3] split along the 3T axis.

What remains per batch (B=16, C4=128 tokens, T=D=512):
  tm1 = TM(x)   = x + kan64->512(kan512->64(LN(x)))
  y1  = gelu(tm1 @ W1f.T + b1)
  cm  = kan512->512(x)
  tm2 = TM(cm)
  y2  = gelu(tm2 @ W2f.T + b2)
  out = y1 + kan512->512(y2)

kan(x) = silu(x) @ Wb.T + bspl(x) (.) Ws.  The 8 cubic B-spline bases are
bumps B(u - g) with u = 2.5x (centered grid constants c_g = g - 3.5):
  e = |u - c_g|, v' = min(e - 2, 0), basis = BUMP3N(v') = v^3 - 4 relu(v-1)^3
with v = -v'.  e comes from Act (AF.Abs with scale/bias) or a DVE L/R
min-pair; v' is one packed 4x-mode TensorScalar; BUMP3N is one packed
custom DVE op (8 ALU stages) over all 8 planes.

Everything is fp16 (activations+weights), fp32 PSUM.  Residual adds
(x + kan, y1 + k2) are identity-matmul accumulations on PE.  Layernorm
applies come for free: u_A = xT*(2.5*rstd) - 2.5*mu*rstd via one
ptr-scalar TS op per batch.  Single act-function table set
(sigmoid_and_others: Abs/Sigmoid/Erf/Square/Identity/Copy).

Sharding: data-parallel over batch, 2 batches per core on 8 cores, weights
replicated.  Activations live transposed (features on partitions, 256 =
2x128 tokens on the free axis).
"""
from contextlib import ExitStack

import numpy as np

import concourse.bacc as bacc
import concourse.bass as bass
import concourse.mybir as mybir
import concourse.tile as tile
from concourse.bass import ts
from concourse.bass_utils import run_bass_kernel_spmd
from concourse.masks import make_identity

import concourse.dve_ops as dve_ops
from concourse.dve_ops import DveOp
from concourse.dve_spec import Spec, Src0, Src1, C0, C1, One, relu, sq, lower
from concourse.dve_uop import DveOpSpec

F16 = mybir.dt.float16
F32 = mybir.dt.float32
AF = mybir.ActivationFunctionType
ALU = mybir.AluOpType

B, C4, T = 16, 128, 512
NCORES = 8
BPC = B // NCORES          # batches per core
NTOK = BPC * C4            # 256 tokens on the free axis
INV_CNT = 1.0 / (C4 * T)
EPS = 1e-5
ISQ2 = float(1.0 / np.sqrt(2.0))
CS = [float(g) - 3.5 for g in range(8)]   # centered grid offsets (u = 2.5x)
CHUNK = 512                # free-axis chunk (of 4*NTOK = 1024) for big sets
NCH = (4 * NTOK) // CHUNK
NEWTON_ITERS = 6

# e-planes of the big sets all come from Act Abs; v' of planes [0, NPOOL)
# runs on Pool (sub,min), the rest packed on DVE (4x mode).
NPOOL = 2

_COMPILED = {}


# --------------------------------------------------------------------------
# custom DVE ops
# --------------------------------------------------------------------------
def _reg(name, spec):
    have = {op.name for op in dve_ops.OPS}
    if name in have:
        return next(op for op in dve_ops.OPS if op.name == name)
    shas = {}
    for ver in ("v3", "v4"):
        s = DveOpSpec(name=name, opcode=0, uops=lower(spec, ver=ver))
        shas[ver] = s.sha(ver)
    op = DveOp(name, spec, subdim=False, uops_sha=shas)
    dve_ops.OPS.append(op)
    dve_ops._SUB_OPCODE_FOR_NAME[name] = (
        dve_ops._CUSTOM_DVE_ROW_BASE + len(dve_ops.OPS) - 1
    )
    dve_ops.CUSTOM_DVE_SPECS[name] = spec
    return op


_r = relu(C1 - Src0)
# in0 = v' = min(e-2, 0) <= 0; with s0=-4, s1=-1 computes v^3 - 4 relu(v-1)^3
BUMP3N = _reg(
    "KAN_BUMP3N",
    Spec(
        body=(sq(_r) * _r) * C0 - sq(Src0) * Src0,
        reference=lambda in0, in1, s0, s1, imm2: s0
        * np.maximum(s1 - in0, 0.0) ** 3
        - in0**3,
    ),
)
# one Newton-rsqrt step: out = Src1*(C0 - (Src0*sq(Src1))*C1), C0=1.5 C1=0.5
RSQRT_NR = _reg(
    "KAN_RSQRT_NR",
    Spec(
        body=(C0 - (Src0 * sq(Src1)) * C1) * Src1,
        reference=lambda in0, in1, s0, s1, imm2: (s0 - in0 * in1 * in1 * s1)
        * in1,
    ),
)
# gelu finish: ((psum + b) * 0.5) * (erf + 1)
GELU_FIN2 = _reg(
    "GELU_FIN2",
    Spec(
        body=((Src0 + C1) * C0) * (Src1 + One),
        reference=lambda in0, in1, s0, s1, imm2: ((in0 + s1) * s0)
        * (in1 + 1.0),
    ),
)


# --------------------------------------------------------------------------
# kernel builder
# --------------------------------------------------------------------------
class _KB:
    def __init__(self, nc, tc, ctx):
        self.nc = nc
        self.tc = tc
        p = lambda **kw: ctx.enter_context(tc.tile_pool(**kw))
        self.singles = p(name="singles", bufs=1)
        self.act = p(name="act", bufs=1)      # fp16 activation planes
        self.actf = p(name="actf", bufs=1)    # fp32 planes (xN/outN/oT)
        self.ev = p(name="ev", bufs=2)        # e chunk tiles (Act->DVE)
        self.feat = p(name="feat", bufs=2)    # feature chunk tiles
        self.sfeat = p(name="sfeat", bufs=1)  # small-set tiles
        self.scr = p(name="scr", bufs=2)      # fp32 scratch (erf)
        self.tiny = p(name="tiny", bufs=8)
        self.psum4 = p(name="psum4", bufs=4, space="PSUM")
        self.psum = p(name="psum", bufs=3, space="PSUM")
        self.psum1 = p(name="psum1", bufs=1, space="PSUM")

        self.identf = self.singles.tile([128, 128], F32)
        make_identity(nc, self.identf[:])
        self.identh = self.singles.tile([128, 128], F16)
        make_identity(nc, self.identh[:])
        self.ones = self.singles.tile([128, 128], F32)
        nc.gpsimd.memset(self.ones[:], 1.0)
        # per-basis Act bias (-c_g) and pair bias for the small sets
        self.cb = self.singles.tile([128, 8], F32)
        for g in range(8):
            nc.gpsimd.memset(self.cb[:, g : g + 1], -CS[g])
        self.cbp = self.singles.tile([128, 4], F32)
        for j in range(4):
            nc.gpsimd.memset(self.cbp[0:64, j : j + 1], -CS[2 * j])
            nc.gpsimd.memset(self.cbp[64:128, j : j + 1], -CS[2 * j + 1])
        # pin the act-table set (contains Sigmoid+Abs+Erf+Identity) at t~0 so
        # the 1.3us table load hides under the input DMA
        warm = self.singles.tile([128, 1], F32, name="warm")
        nc.scalar.activation(warm[:], self.ones[:, 0:1], AF.Sigmoid)

    def pmid(self, name):
        """Shared [128,256] psum ring (transposes, tm-k2, gcn, stat folds)."""
        return self.psum.tile([128, NTOK], F32, tag="pmid", name=name)

    # ---- big feature set, one chunk of the free axis --------------------- #
    def feat_chunk(self, src, scale, feat, W, lr=(), ub=None):
        """src: fp16 AP with W free elems.  scale: 2.5 if src is raw (x/y2),
        1.0 if src is already u = 2.5*z.  feat: (128,9,W) AP; planes 0-7
        bases, plane 8 silu.  lr: planes computed via Pool L/R min-pair
        (requires scale==1.0 and ub = -src); the rest use Act Abs."""
        nc = self.nc
        na = 8 - len(lr)           # act-e planes [0, na)
        e_t = self.ev.tile([128, 8, CHUNK], F16, tag="e")
        e = e_t[:, :, :W]
        for g in range(na):
            nc.scalar.activation(
                e[:, g, :], src, AF.Abs, bias=self.cb[:, g : g + 1],
                scale=scale,
            )
        vp_t = self.singles.tile([128, 8, CHUNK], F16, tag="vp")
        vp = vp_t[:, :, :W]
        npool = NPOOL if not lr else 0
        if npool:
            nc.gpsimd.tensor_scalar(
                out=vp[:, 0:npool, :], in0=e[:, 0:npool, :], scalar1=2.0,
                scalar2=0.0, op0=ALU.subtract, op1=ALU.min,
            )
        if npool < 2:
            nc.vector.tensor_scalar(
                out=vp[:, npool:2, :], in0=e[:, npool:2, :], scalar1=2.0,
                scalar2=0.0, op0=ALU.subtract, op1=ALU.min,
            )
        nc.vector._custom_dve(
            BUMP3N, out=feat[:, 0:2, :], in0=vp[:, 0:2, :], s0=-4.0, s1=-1.0
        )
        for h in (1, 2, 3):
            if 2 * h + 2 <= na:
                nc.vector.tensor_scalar(
                    out=vp[:, 2 * h : 2 * h + 2, :],
                    in0=e[:, 2 * h : 2 * h + 2, :], scalar1=2.0,
                    scalar2=0.0, op0=ALU.subtract, op1=ALU.min,
                )
            if h < 3:
                nc.vector._custom_dve(
                    BUMP3N, out=feat[:, 2 * h : 2 * h + 2, :],
                    in0=vp[:, 2 * h : 2 * h + 2, :], s0=-4.0, s1=-1.0,
                )
        if lr:
            assert scale == 1.0 and ub is not None
            nlr = len(lr)
            Lt_t = self.singles.tile([128, 4, CHUNK], F16, tag="Lt")
            Lt = Lt_t[:, :nlr, :W]
            for i, g in enumerate(lr):
                nc.vector.tensor_scalar(
                    out=Lt[:, i, :], in0=src, scalar1=CS[g] + 2.0,
                    scalar2=0.0, op0=ALU.subtract, op1=ALU.min,
                )
                nc.vector.tensor_scalar(
                    out=vp[:, g, :], in0=ub, scalar1=CS[g] - 2.0,
                    scalar2=0.0, op0=ALU.add, op1=ALU.min,
                )
            nc.vector.tensor_tensor(
                out=vp[:, na:8, :], in0=vp[:, na:8, :], in1=Lt, op=ALU.max,
            )
        nc.vector._custom_dve(
            BUMP3N, out=feat[:, 6:8, :], in0=vp[:, 6:8, :], s0=-4.0, s1=-1.0
        )
        sg_t = self.ev.tile([128, CHUNK], F16, tag="sg")
        sg = sg_t[:, :W]
        nc.scalar.activation(sg, src, AF.Sigmoid, scale=scale * 0.4)
        nc.gpsimd.tensor_tensor(out=feat[:, 8, :], in0=src, in1=sg,
                                op=ALU.mult)

    # ---- kan matmuls over chunked features -------------------------------- #
    def kan_512(self, w, srcgen, pms, extra=None):
        """w (128,9,4,4,128).  srcgen(k) -> feat chunk tile (128,9,CHUNK)
        covering k-tiles [k*CHUNK/NTOK, ...).  pms: 4 psum tiles (128,NTOK).
        extra: optional (lhsT, rhs_m_list) accumulated at the end (residual).
        Caller closes accumulation via the stop flag on the last matmul."""
        nc = self.nc
        kpc = CHUNK // NTOK  # k-tiles per chunk
        for c in range(NCH):
            f = srcgen(c)
            for kk in range(kpc):
                k = c * kpc + kk
                for g in range(9):
                    for m in range(4):
                        nc.tensor.matmul(
                            pms[m][:], w[:, k, g, m, :],
                            f[:, g, ts(kk, NTOK)],
                            start=(k == 0 and g == 0),
                            stop=(extra is None and k == 3 and g == 8),
                        )
        if extra is not None:
            lhsT, rhs_list = extra
            for m in range(4):
                nc.tensor.matmul(
                    pms[m][:], lhsT, rhs_list[m], start=False, stop=True
                )

    def kan_to64(self, w, featgen, pm):
        """w (128,9,4,128) with duplicated 64-out blocks -> pm (128, NTOK)."""
        nc = self.nc
        kpc = CHUNK // NTOK
        for c in range(NCH):
            f = featgen(c)
            for kk in range(kpc):
                k = c * kpc + kk
                for g in range(9):
                    nc.tensor.matmul(
                        pm[:], w[:, g, k, :], f[:, g, ts(kk, NTOK)],
                        start=(k == 0 and g == 0), stop=(k == 3 and g == 8),
                    )

    def small_set(self, zpsum, tag):
        """zpsum (128, NTOK) psum with rows 64-127 duplicating 0-63 (the
        kan hidden z).  Returns (featp (128,4,NTOK) pair-packed bases,
        sil (64, NTOK))."""
        nc = self.nc
        e = self.sfeat.tile([128, 4, NTOK], F16, tag="se")
        for j in range(4):
            nc.scalar.activation(
                e[:, j, :], zpsum, AF.Abs, bias=self.cbp[:, j : j + 1],
                scale=2.5,
            )
        vp = self.sfeat.tile([128, 4, NTOK], F16, tag="svp")
        featp = self.sfeat.tile([128, 4, NTOK], F16, tag="sf")
        for h in range(2):
            nc.vector.tensor_scalar(
                out=vp[:, 2 * h : 2 * h + 2, :], in0=e[:, 2 * h : 2 * h + 2, :],
                scalar1=2.0, scalar2=0.0, op0=ALU.subtract, op1=ALU.min,
            )
            nc.vector._custom_dve(
                BUMP3N, out=featp[:, 2 * h : 2 * h + 2, :],
                in0=vp[:, 2 * h : 2 * h + 2, :], s0=-4.0, s1=-1.0,
            )
        sg = self.sfeat.tile([64, NTOK], F16, tag="ssg")
        nc.scalar.activation(sg[:], zpsum[0:64, :], AF.Sigmoid)
        sil = self.sfeat.tile([64, NTOK], F16, tag="ssil")
        nc.vector.tensor_tensor(out=sil[:], in0=zpsum[0:64, :], in1=sg[:],
                                op=ALU.mult)
        return featp, sil

    # ---- layernorm scalars (all-DVE tail: minimal cross-engine latency) - #
    def _ln_tail(self, sF, nm):
        """sF (128, 2*BPC) [mu, e2] pairs -> (ys, cs) = 2.5*rstd, 2.5*mu*rstd."""
        nc = self.nc
        mu = sF[:, 0 : 2 * BPC : 2]
        e2 = sF[:, 1 : 2 * BPC : 2]
        var = self.tiny.tile([128, BPC], F32, name=f"var{nm}")
        nc.vector.tensor_tensor(out=var[:], in0=mu, in1=mu, op=ALU.mult)
        nc.vector.tensor_tensor(out=var[:], in0=e2, in1=var[:],
                                op=ALU.subtract)
        a = self.tiny.tile([128, BPC], F32, name=f"a{nm}")
        nc.vector.tensor_scalar(out=a[:], in0=var[:], scalar1=EPS,
                                scalar2=None, op0=ALU.add)
        y = self.tiny.tile([128, BPC], F32, name=f"y{nm}")
        nc.vector.reciprocal(y[:], a[:])
        nc.vector.tensor_scalar(out=y[:], in0=y[:], scalar1=1.0, scalar2=None,
                                op0=ALU.min)
        for _ in range(4):
            nc.vector._custom_dve(RSQRT_NR, out=y[:], in0=a[:], in1=y[:],
                                  s0=1.5, s1=0.5)
        ys = self.tiny.tile([128, BPC], F32, name=f"ys{nm}")
        nc.vector.tensor_scalar(out=ys[:], in0=y[:], scalar1=2.5,
                                scalar2=None, op0=ALU.mult)
        cs = self.tiny.tile([128, BPC], F32, name=f"cs{nm}")
        nc.vector.tensor_tensor(out=cs[:], in0=mu, in1=ys[:], op=ALU.mult)
        return ys, cs

    def ln_scalars(self, stats, smap, nslots):
        """stats (128, nslots): [sum, sq] pair per batch (nslots == 2*BPC)."""
        nc = self.nc
        pstat_t = self.pmid("pstat")
        pstat = pstat_t[:, :nslots]
        nc.tensor.matmul(pstat, self.ones[:], stats[:, :nslots],
                         start=True, stop=True)
        sG = self.tiny.tile([128, nslots], F32, name="sG")
        nc.vector.tensor_scalar(
            out=sG[:], in0=pstat, scalar1=INV_CNT, scalar2=None,
            op0=ALU.mult,
        )
        return self._ln_tail(sG, "1")

    def make_u(self, srcf16, ys, cs, tag, lnw=None, lnb=None):
        """u = src*(2.5 rstd) - 2.5 mu rstd per batch; optional affine."""
        nc = self.nc
        u = self.act.tile([128, 4, NTOK], F16, tag="uln", name=tag)
        for b in range(BPC):
            nc.vector.tensor_scalar(
                out=u[:, :, ts(b, C4)], in0=srcf16[:, :, ts(b, C4)],
                scalar1=ys[:, b : b + 1], scalar2=cs[:, b : b + 1],
                op0=ALU.mult, op1=ALU.subtract,
            )
        if lnw is not None:
            nc.vector.tensor_tensor(out=u[:], in0=u[:], in1=lnw[:],
                                    op=ALU.mult)
        if lnb is not None:
            nc.vector.tensor_tensor(out=u[:], in0=u[:], in1=lnb[:],
                                    op=ALU.add)
        return u

    # ---- gcn -------------------------------------------------------------- #
    def gcn(self, tm, wg, bias2, yname, ms=(0, 1, 2, 3), y=None):
        """tm (128,4,NTOK) fp16; wg (128,4,4,128) fp16; bias2 (128,4,2) f32
        cols [b, b*isq2].  Returns y (128,4,NTOK) fp16."""
        nc = self.nc
        if y is None:
            y = self.act.tile([128, 4, NTOK], F16, tag=yname, name=yname)
        for m in ms:
            pm = self.pmid(f"pg{m}")
            for k in range(4):
                nc.tensor.matmul(
                    pm[:], wg[:, k, m, :], tm[:, k, :],
                    start=(k == 0), stop=(k == 3),
                )
            er = self.scr.tile([128, NTOK], F32, tag="erf")
            nc.scalar.activation(
                er[:], pm[:], AF.Erf, bias=bias2[:, m, 1:2], scale=ISQ2
            )
            nc.vector._custom_dve(
                GELU_FIN2, out=y[:, m, :], in0=pm[:], in1=er[:], s0=0.5,
                s1=bias2[:, m, 0:1],
            )
        return y


def _emit(nc, ln_flags):
    use_lnw1, use_lnb1, use_lnw2, use_lnb2 = ln_flags
    dram = {}

    def din(name, shape, dt=F16):
        dram[name] = nc.dram_tensor(name, shape, dt, kind="ExternalInput").ap()
        return dram[name]

    x_d = din("x_sh", (BPC, C4, T), F32)
    w_k1 = din("w_k1", (128, 4, 9, 4, 128))
    w_tm1k1 = din("w_tm1k1", (128, 9, 4, 128))
    w_tm1k2 = din("w_tm1k2", (128, 4, 4, 128))
    w_tm1k2s = din("w_tm1k2s", (64, 4, 128))
    w_g1 = din("w_g1", (128, 4, 4, 128))
    b_g1 = din("b_g1", (128, 4, 2), F32)
    w_tm2k1 = din("w_tm2k1", (128, 9, 4, 128))
    w_tm2k2 = din("w_tm2k2", (128, 4, 4, 128))
    w_tm2k2s = din("w_tm2k2s", (64, 4, 128))
    w_g2 = din("w_g2", (128, 4, 4, 128))
    b_g2 = din("b_g2", (128, 4, 2), F32)
    w_k2 = din("w_k2", (128, 4, 9, 4, 128))
    ln1w_d = din("ln1w", (128, 4, NTOK)) if use_lnw1 else None
    ln1b_d = din("ln1b", (128, 4, NTOK)) if use_lnb1 else None
    ln2w_d = din("ln2w", (128, 4, NTOK)) if use_lnw2 else None
    ln2b_d = din("ln2b", (128, 4, NTOK)) if use_lnb2 else None
    out_d = nc.dram_tensor("out_sh", (BPC, C4, T), F32,
                           kind="ExternalOutput").ap()

    with tile.TileContext(nc) as tc, ExitStack() as ctx:
        kb = _KB(nc, tc, ctx)
        wpool = ctx.enter_context(tc.tile_pool(name="weights", bufs=1))
        sync = nc.sync

        def wload(ap, shape, tag, dt=F16):
            t = wpool.tile(list(shape), dt, tag=tag)
            sync.dma_start(t[:], ap)
            return t

        # ---- input + weight DMA (consumption order) ----
        xN = kb.actf.tile([C4, BPC, T], F32, tag="xN")
        x_r = x_d.rearrange("b p t -> p b t")
        for k in range(4):
            for b in range(BPC):
                sync.dma_start(xN[:, b, ts(k, 128)], x_r[:, b, ts(k, 128)])
        W_k1 = wpool.tile([128, 4, 9, 4, 128], F16, tag="wk_big")
        for k in range(4):
            sync.dma_start(W_k1[:, k], w_k1[:, k])
        W_tm1k1 = wload(w_tm1k1, (128, 9, 4, 128), "wtmk1")
        W_tm1k2 = wload(w_tm1k2, (128, 4, 4, 128), "wtmk2")
        W_tm1k2s = wload(w_tm1k2s, (64, 4, 128), "wtmk2s")
        W_g1 = wload(w_g1, (128, 4, 4, 128), "wg")
        B_g1 = wload(b_g1, (128, 4, 2), "bg", F32)
        W_tm2k1 = wload(w_tm2k1, (128, 9, 4, 128), "wtm2k1")
        W_tm2k2 = wload(w_tm2k2, (128, 4, 4, 128), "wtm2k2")
        W_tm2k2s = wload(w_tm2k2s, (64, 4, 128), "wtm2k2s")
        W_g2 = wload(w_g2, (128, 4, 4, 128), "wg2")
        B_g2 = wload(b_g2, (128, 4, 2), "bg2", F32)
        W_k2 = wpool.tile([128, 4, 9, 4, 128], F16, tag="wk_big2")
        for k in range(4):
            sync.dma_start(W_k2[:, k], w_k2[:, k])
        LN1W = wload(ln1w_d, (128, 4, NTOK), "lnw") if use_lnw1 else None
        LN1B = wload(ln1b_d, (128, 4, NTOK), "lnb") if use_lnb1 else None
        LN2W = wload(ln2w_d, (128, 4, NTOK), "ln2w") if use_lnw2 else None
        LN2B = wload(ln2b_d, (128, 4, NTOK), "ln2b") if use_lnb2 else None

        # ---- transpose x -> xT fp16 (copies on DVE) ----
        xT = kb.act.tile([128, 4, NTOK], F16, tag="xT")
        for k in range(4):
            for b in range(BPC):
                pt = kb.pmid(f"ptr_{k}_{b}")
                nc.tensor.transpose(pt[:, 0:128], xN[:, b, ts(k, 128)],
                                    kb.identf[:])
                nc.vector.tensor_scalar(
                    out=xT[:, k, ts(b, 128)], in0=pt[:, 0:128], scalar1=1.0,
                    scalar2=None, op0=ALU.mult,
                )

        # chunk lists: (k0, nk) over the 4 k-tiles
        CH_X = [(0, 1), (1, 1), (2, 2)]
        CH_A = [(0, 2), (2, 2)]
        CH_Y = [(0, 1), (1, 1), (2, 1), (3, 1)]

        def gen(src_tile, scale, cache, ci, k0, nk, lr=(), ub_tile=None):
            if ci not in cache:
                f = kb.feat.tile([128, 9, CHUNK], F16, tag="fch",
                                 name=f"f{len(cache)}")
                ub = ub_tile[:, k0 : k0 + nk, :] if ub_tile is not None \
                    else None
                kb.feat_chunk(src_tile[:, k0 : k0 + nk, :], scale,
                              f[:, :, : nk * NTOK], nk * NTOK, lr=lr, ub=ub)
                cache[ci] = f
            return cache[ci]

        def kan512_mms(w, pms, f, k0, nk, extra_rhs=None, close=False):
            for kk in range(nk):
                k = k0 + kk
                for g in range(9):
                    for m in range(4):
                        nc.tensor.matmul(
                            pms[m][:], w[:, k, g, m, :],
                            f[:, g, ts(kk, NTOK)],
                            start=(k == 0 and g == 0),
                            stop=(close and extra_rhs is None
                                  and k == 3 and g == 8),
                        )
            if extra_rhs is not None:
                for m in range(4):
                    nc.tensor.matmul(pms[m][:], kb.identh[:], extra_rhs[m],
                                     start=False, stop=True)

        def kan64_mms(w, pm, f, k0, nk, close):
            for kk in range(nk):
                k = k0 + kk
                for g in range(9):
                    nc.tensor.matmul(
                        pm[:], w[:, g, k, :], f[:, g, ts(kk, NTOK)],
                        start=(k == 0 and g == 0),
                        stop=(close and k == 3 and g == 8),
                    )

        # ================= phase 1: featX + k1 (+ featA during) ==========
        featX, featA, featC, featY = {}, {}, {}, {}
        pk1 = [
            kb.psum4.tile([128, NTOK], F32, tag="pk1", name=f"pk1_{m}")
            for m in range(4)
        ]
        gen(xT, 2.5, featX, 0, *CH_X[0])
        gen(xT, 2.5, featX, 1, *CH_X[1])
        kan512_mms(W_k1, pk1, featX[0], *CH_X[0])
        kan512_mms(W_k1, pk1, featX[1], *CH_X[1])
        gen(xT, 2.5, featX, 2, *CH_X[2])
        # LN1 stats: sums on Act, squares on DVE (split the latency)
        st1 = kb.tiny.tile([128, 4], F32, name="st1")
        for b in range(BPC):
            s1 = kb.singles.tile([C4, T], F32, tag="ascr", name=f"sa{b}")
            nc.vector.tensor_scalar(
                out=s1[:], in0=xN[:, b, :], scalar1=1.0, scalar2=0.0,
                op0=ALU.mult, op1=ALU.add,
                accum_out=st1[:, 2 * b : 2 * b + 1],
            )
            s2 = kb.singles.tile([C4, T], F32, tag="ascr", name=f"sb{b}")
            nc.vector.scalar_tensor_tensor(
                out=s2[:], in0=xN[:, b, :], scalar=1.0, in1=xN[:, b, :],
                op0=ALU.mult, op1=ALU.mult,
                accum_out=st1[:, 2 * b + 1 : 2 * b + 2],
            )
        # LN1 fold lands in the PE stream here (st1 ready by now)
        ys1, cs1 = kb.ln_scalars(st1, [0, 1], 4)
        u_A = kb.make_u(xT, ys1, cs1, "uA", LN1W, LN1B)
        gen(u_A, 1.0, featA, 0, *CH_A[0])
        gen(u_A, 1.0, featA, 1, *CH_A[1])

        # ---- k1 tail m-major: close each m-tile early; cm copies (DVE,
        #      with LN2 sum accumulation) stream out per m-tile ----
        cm = kb.act.tile([128, 4, NTOK], F16, tag="cm")
        st2 = kb.tiny.tile([128, 12], F32, name="st2")
        smap2 = [b for b in range(BPC) for _ in range(4)] + list(range(BPC))
        k0, nk = CH_X[2]
        for m in range(4):
            for kk in range(nk):
                k = k0 + kk
                for g in range(9):
                    nc.tensor.matmul(
                        pk1[m][:], W_k1[:, k, g, m, :],
                        featX[2][:, g, ts(kk, NTOK)],
                        start=False, stop=(k == 3 and g == 8),
                    )
            for b in range(BPC):
                j = b * 4 + m
                nc.vector.tensor_scalar(
                    out=cm[:, m, ts(b, 128)], in0=pk1[m][:, ts(b, 128)],
                    scalar1=1.0, scalar2=0.0, op0=ALU.mult, op1=ALU.add,
                    accum_out=st2[:, j : j + 1],
                )
        for b in range(BPC):
            sq_s = kb.singles.tile([128, 4, 128], F32, tag="ascr2",
                                   name=f"sq{b}")
            nc.vector.scalar_tensor_tensor(
                out=sq_s[:], in0=cm[:, :, ts(b, C4)], scalar=1.0,
                in1=cm[:, :, ts(b, C4)], op0=ALU.mult, op1=ALU.mult,
                accum_out=st2[:, 8 + b : 9 + b],
            )
        ys2, cs2 = kb.ln_scalars_mixed(st2, smap2)
        u_C = kb.make_u(cm, ys2, cs2, "uC", LN2W, LN2B)

        pz1 = kb.psum1.tile([128, NTOK], F32, tag="pz", name="pz1")
        kan64_mms(W_tm1k1, pz1, featA[0], *CH_A[0], close=False)
        kan64_mms(W_tm1k1, pz1, featA[1], *CH_A[1], close=True)

        # featC gates tm2k1 (critical)
        gen(u_C, 1.0, featC, 0, 0, 2)
        gen(u_C, 1.0, featC, 1, 2, 2)

        featB, silB = kb.small_set(pz1[:], "B")
        tm1 = kb.act.tile([128, 4, NTOK], F16, tag="tm", name="tm1")
        ptm1 = []
        for m in range(4):
            pm = kb.psum4.tile([128, NTOK], F32, tag="pk1", name=f"pt1_{m}")
            for j in range(4):
                nc.tensor.matmul(pm[:], W_tm1k2[:, j, m, :], featB[:, j, :],
                                 start=(j == 0), stop=False)
            nc.tensor.matmul(pm[:], W_tm1k2s[:, m, :], silB[:], start=False,
                             stop=False)
            nc.tensor.matmul(pm[:], kb.identh[:], xT[:, m, :], start=False,
                             stop=True)
            ptm1.append(pm)
        for m in range(4):
            nc.scalar.copy(tm1[:, m, :], ptm1[m][:])
        y1 = kb.gcn(tm1, W_g1, B_g1, "y1")

        # ================= TM2 head =====================================
        pz2 = kb.psum1.tile([128, NTOK], F32, tag="pz", name="pz2")
        kan64_mms(W_tm2k1, pz2, featC[0], 0, 2, close=False)
        kan64_mms(W_tm2k1, pz2, featC[1], 2, 2, close=True)
        featD, silD = kb.small_set(pz2[:], "D")
        tm2 = kb.act.tile([128, 4, NTOK], F16, tag="tm", name="tm2")
        ptm2 = []
        for m in range(4):
            pm = kb.psum4.tile([128, NTOK], F32, tag="pk1", name=f"pt2_{m}")
            for j in range(4):
                nc.tensor.matmul(pm[:], W_tm2k2[:, j, m, :], featD[:, j, :],
                                 start=(j == 0), stop=False)
            nc.tensor.matmul(pm[:], W_tm2k2s[:, m, :], silD[:], start=False,
                             stop=False)
            nc.tensor.matmul(pm[:], kb.identh[:], cm[:, m, :], start=False,
                             stop=True)
            ptm2.append(pm)
        for m in range(4):
            nc.vector.tensor_scalar(
                out=tm2[:, m, :], in0=ptm2[m][:], scalar1=1.0, scalar2=None,
                op0=ALU.mult,
            )

        # ---- gcn2 interleaved with featY (chunk m needs y2 plane m) ----
        y2 = kb.gcn(tm2, W_g2, B_g2, "y2", ms=(0,))
        gen(y2, 2.5, featY, 0, *CH_Y[0])
        kb.gcn(tm2, W_g2, B_g2, "y2", ms=(1,), y=y2)
        gen(y2, 2.5, featY, 1, *CH_Y[1])
        kb.gcn(tm2, W_g2, B_g2, "y2", ms=(2,), y=y2)
        gen(y2, 2.5, featY, 2, *CH_Y[2])
        kb.gcn(tm2, W_g2, B_g2, "y2", ms=(3,), y=y2)
        gen(y2, 2.5, featY, 3, *CH_Y[3])

        # ================= k2 + y1 residual + out =======================
        pk2 = [
            kb.psum4.tile([128, NTOK], F32, tag="pk1", name=f"pk2_{m}")
            for m in range(4)
        ]
        kan512_mms(W_k2, pk2, featY[0], *CH_Y[0])
        kan512_mms(W_k2, pk2, featY[1], *CH_Y[1])
        kan512_mms(W_k2, pk2, featY[2], *CH_Y[2])

        outN = kb.actf.tile([C4, BPC, T], F32, tag="outN")
        out_r = out_d.rearrange("b p t -> p b t")
        # last chunk m-major: close each m-tile early and stream its
        # transpose + store while the next m-tile accumulates
        for m in range(4):
            for g in range(9):
                nc.tensor.matmul(
                    pk2[m][:], W_k2[:, 3, g, m, :], featY[3][:, g, 0:NTOK],
                    start=False, stop=False,
                )
            nc.tensor.matmul(pk2[m][:], kb.identh[:], y1[:, m, :],
                             start=False, stop=True)
            oT = kb.actf.tile([128, NTOK], F32, tag="oT", name=f"oT{m}")
            nc.scalar.copy(oT[:], pk2[m][:])
            for b in range(BPC):
                pt = kb.pmid(f"ptro_{m}_{b}")
                nc.tensor.transpose(pt[:, 0:128], oT[:, ts(b, 128)],
                                    kb.identf[:])
                nc.scalar.copy(outN[:, b, ts(m, 128)], pt[:, 0:128])
            sync.dma_start(out_r[:, :, ts(m, 128)], outN[:, :, ts(m, 128)])

    return dram


# mixed-slot LN fold (4 sum slots + 1 sq slot per batch)
def _ln_scalars_mixed(self, stats, smap):
    nc = self.nc
    nslots = len(smap)
    pstat_t = self.pmid("pstat2")
    pstat = pstat_t[:, :nslots]
    nc.tensor.matmul(pstat, self.ones[:], stats[:, :nslots], start=True,
                     stop=True)
    sG = self.tiny.tile([128, nslots], F32, name="sG2")
    nc.vector.tensor_scalar(out=sG[:], in0=pstat, scalar1=INV_CNT,
                            scalar2=None, op0=ALU.mult)
    # slots: b*4+m sums, 8+b sq -> build [mu, e2] pairs
    sF = self.tiny.tile([128, 2 * BPC], F32, name="sF2")
    for b in range(BPC):
        nc.vector.tensor_tensor(
            out=sF[:, 2 * b : 2 * b + 1], in0=sG[:, 4 * b : 4 * b + 1],
            in1=sG[:, 4 * b + 1 : 4 * b + 2], op=ALU.add,
        )
        nc.vector.tensor_tensor(
            out=sF[:, 2 * b + 1 : 2 * b + 2], in0=sG[:, 4 * b + 2 : 4 * b + 3],
            in1=sG[:, 4 * b + 3 : 4 * b + 4], op=ALU.add,
        )
        nc.vector.tensor_tensor(
            out=sF[:, 2 * b : 2 * b + 1], in0=sF[:, 2 * b : 2 * b + 1],
            in1=sF[:, 2 * b + 1 : 2 * b + 2], op=ALU.add,
        )
        nc.vector.tensor_scalar(
            out=sF[:, 2 * b + 1 : 2 * b + 2], in0=sG[:, 8 + b : 9 + b],
            scalar1=1.0, scalar2=None, op0=ALU.mult,
        )
    return self._ln_tail(sF, "2")


_KB.ln_scalars_mixed = _ln_scalars_mixed


def _build(ln_flags):
    if ln_flags in _COMPILED:
        return _COMPILED[ln_flags]
    nc = bacc.Bacc("TRN2", target_bir_lowering=False, debug=False)
    _emit(nc, ln_flags)
    nc.compile()
    _COMPILED[ln_flags] = nc
    return nc


# --------------------------------------------------------------------------
# host-side weight preparation (all fp16)
# --------------------------------------------------------------------------
def _prep_kan_512(base_w, spline_w):
    """-> (128, 4, 9, 4, 128): [p, ktile, plane, mtile, out]."""
    w = np.empty((128, 4, 9, 4, 128), np.float32)
    for k in range(4):
        for m in range(4):
            blk = spline_w[m * 128 : (m + 1) * 128, k * 128 : (k + 1) * 128, :]
            w[:, k, 0:8, m, :] = blk.transpose(1, 2, 0) / 6.0
            w[:, k, 8, m, :] = (
                base_w[m * 128 : (m + 1) * 128, k * 128 : (k + 1) * 128].T
            )
    return np.ascontiguousarray(w.astype(np.float16))


def _prep_kan_to64(base_w, spline_w):
    """base (64,512) spline (64,512,8) -> (128, 9, 4, 128) with the 64 output
    rows duplicated; silu plane scaled by 0.4 (sil plane = u*sigmoid(0.4u) =
    2.5*silu(z))."""
    w = np.empty((128, 9, 4, 128), np.float32)
    for k in range(4):
        blk = spline_w[:, k * 128 : (k + 1) * 128, :]  # (64, 128, 8)
        pl = blk.transpose(1, 2, 0) / 6.0              # (128, 8, 64)
        w[:, 0:8, k, 0:64] = pl
        w[:, 0:8, k, 64:128] = pl
        bb = base_w[:, k * 128 : (k + 1) * 128].T * 0.4
        w[:, 8, k, 0:64] = bb
        w[:, 8, k, 64:128] = bb
    return np.ascontiguousarray(w.astype(np.float16))


def _prep_kan_from64(base_w, spline_w):
    """base (512,64) spline (512,64,8) -> pair-packed (128,4,4,128) +
    silu (64,4,128)."""
    wp = np.empty((128, 4, 4, 128), np.float32)
    ws = np.empty((64, 4, 128), np.float32)
    for m in range(4):
        blk = spline_w[m * 128 : (m + 1) * 128, :, :]  # (128out, 64in, 8)
        for j in range(4):
            wp[0:64, j, m, :] = blk[:, :, 2 * j].T / 6.0
            wp[64:128, j, m, :] = blk[:, :, 2 * j + 1].T / 6.0
        ws[:, m, :] = base_w[m * 128 : (m + 1) * 128, :].T
    return (
        np.ascontiguousarray(wp.astype(np.float16)),
        np.ascontiguousarray(ws.astype(np.float16)),
    )


def _prep_gcn(gw, gb):
    Wf = gw[:, :512] + gw[:, 512:1024] + gw[:, 1024:]
    w = np.empty((128, 4, 4, 128), np.float32)
    for k in range(4):
        for m in range(4):
            w[:, k, m, :] = Wf[m * 128 : (m + 1) * 128,
                               k * 128 : (k + 1) * 128].T
    b = np.empty((128, 4, 2), np.float32)
    b[:, :, 0] = gb.reshape(4, 128).T
    b[:, :, 1] = b[:, :, 0] * ISQ2
    return np.ascontiguousarray(w.astype(np.float16)), np.ascontiguousarray(b)


def _ln_plane(a, scale=1.0):
    """ln param (512, 128) -> (128, 4, NTOK) fp16 duplicated over batches."""
    p = np.empty((128, 4, NTOK), np.float32)
    for k in range(4):
        for b in range(BPC):
            p[:, k, b * C4 : (b + 1) * C4] = a[k * 128 : (k + 1) * 128, :] * scale
    return np.ascontiguousarray(p.astype(np.float16))


def kernel(**inputs):
    i = {k: np.asarray(v) for k, v in inputs.items()}
    use_lnw1 = not np.all(i["tm1_ln_w"] == 1.0)
    use_lnb1 = not np.all(i["tm1_ln_b"] == 0.0)
    use_lnw2 = not np.all(i["tm_ln_w"] == 1.0)
    use_lnb2 = not np.all(i["tm_ln_b"] == 0.0)
    ln_flags = (use_lnw1, use_lnb1, use_lnw2, use_lnb2)
    nc = _build(ln_flags)

    w_tm1k2, w_tm1k2s = _prep_kan_from64(i["tm1_k2_base"], i["tm1_k2_spline"])
    w_tm2k2, w_tm2k2s = _prep_kan_from64(i["tm_k2_base"], i["tm_k2_spline"])
    w_g1, b_g1 = _prep_gcn(i["g1_w"], i["g1_b"])
    w_g2, b_g2 = _prep_gcn(i["g2_w"], i["g2_b"])

    shared = dict(
        w_k1=_prep_kan_512(i["k1_base"], i["k1_spline"]),
        w_k2=_prep_kan_512(i["k2_base"], i["k2_spline"]),
        w_tm1k1=_prep_kan_to64(i["tm1_k1_base"], i["tm1_k1_spline"]),
        w_tm2k1=_prep_kan_to64(i["tm_k1_base"], i["tm_k1_spline"]),
        w_tm1k2=w_tm1k2, w_tm1k2s=w_tm1k2s,
        w_tm2k2=w_tm2k2, w_tm2k2s=w_tm2k2s,
        w_g1=w_g1, b_g1=b_g1, w_g2=w_g2, b_g2=b_g2,
    )
    if use_lnw1:
        shared["ln1w"] = _ln_plane(i["tm1_ln_w"])
    if use_lnb1:
        shared["ln1b"] = _ln_plane(i["tm1_ln_b"], 2.5)
    if use_lnw2:
        shared["ln2w"] = _ln_plane(i["tm_ln_w"])
    if use_lnb2:
        shared["ln2b"] = _ln_plane(i["tm_ln_b"], 2.5)
    x = np.ascontiguousarray(i["x"], np.float32)
    in_maps = [
        {"x_sh": x[c * BPC : (c + 1) * BPC], **shared} for c in range(NCORES)
    ]
    res = run_bass_kernel_spmd(nc, in_maps, core_ids=list(range(NCORES)))
    out = np.empty((B, C4, T), np.float32)
    for c in range(NCORES):
        out[c * BPC : (c + 1) * BPC] = res.results[c]["out_sh"]
    return out
